# revision 1
# baseline (speedup 1.0000x reference)
"""MultiHeadLatentAttention TRN2 kernel.

Sharding: 8 cores = 2 (batch) x 4 (head groups of 4 heads).
Each core computes, for its batch b and heads hg*4..hg*4+3:
  - latent down-projections kv_d, q_d (replicated within the batch group)
  - per-head up-projections K^T, Q^T (with RoPE), V
  - full attention for its 4 heads
  - partial output projection (its 512 columns of Wo's input dim)
Partial outputs are summed on the host (+ bo).

All big tensors live in "feature-on-partitions" (transposed) layout so
every matmul has free dim 512 and runs at full fp32r rate.
RoPE's rotate_half is a partition-pair swap: the rope feature rows are
stored in host-permuted order (pairs (i, i+32) adjacent) so DVE
stream_shuffle(mask=i^1) implements the rotation; the sign lives in the
host-built sin table.
Softmax skips max-subtraction (scores are bounded, exp is safe in fp32);
row sums come from a ones-vector matmul, and 1/sum is applied to the
attention output via a gpsimd partition-broadcast + DVE multiply.
"""

import sys

sys.path.insert(0, "/opt/trn_rl_repo")

from contextlib import ExitStack

import numpy as np

H = 16
E = 2048
LAT = E // 4          # 512
D = E // H            # 128
R = D // 2            # 64
B, S = 2, 2048
HPC = H // 4          # 4 heads per core
NCORES = 8
NE = E // 128         # 16 contraction chunks over E
NL = LAT // 128       # 4 contraction chunks over LAT
SW = 512              # s-chunk width for projections
NSC = S // SW         # 4 s-chunks
NKC = S // 128        # 16 key chunks
SCALE = 1.0 / float(np.sqrt(D))

_RT = {}  # cached runtimes


def _mk(nc):
    """Declare DRAM I/O; returns dict of handles."""
    import concourse.mybir as mybir
    F32 = mybir.dt.float32
    F32R = mybir.dt.float32r
    d = {}
    d["xT"] = nc.dram_tensor("xT", [E, S], F32R, kind="ExternalInput")
    d["wkvdT"] = nc.dram_tensor("wkvdT", [E, LAT], F32R, kind="ExternalInput")
    d["wqdT"] = nc.dram_tensor("wqdT", [E, LAT], F32R, kind="ExternalInput")
    d["wrkT"] = nc.dram_tensor("wrkT", [E, HPC * R], F32R,
                               kind="ExternalInput")
    d["wkuT"] = nc.dram_tensor("wkuT", [LAT, HPC * R], F32R,
                               kind="ExternalInput")
    d["wquT"] = nc.dram_tensor("wquT", [LAT, HPC * R], F32R,
                               kind="ExternalInput")
    d["wrqT"] = nc.dram_tensor("wrqT", [LAT, HPC * R], F32R,
                               kind="ExternalInput")
    d["wvuT"] = nc.dram_tensor("wvuT", [LAT, HPC * D], F32R,
                               kind="ExternalInput")
    d["woT"] = nc.dram_tensor("woT", [HPC * D, E], F32R,
                              kind="ExternalInput")
    d["bkvd"] = nc.dram_tensor("bkvd", [128, NL], F32, kind="ExternalInput")
    d["bqd"] = nc.dram_tensor("bqd", [128, NL], F32, kind="ExternalInput")
    d["bku"] = nc.dram_tensor("bku", [128, 2], F32, kind="ExternalInput")
    d["bqu"] = nc.dram_tensor("bqu", [128, 2], F32, kind="ExternalInput")
    d["brk"] = nc.dram_tensor("brk", [128, 2], F32, kind="ExternalInput")
    d["brq"] = nc.dram_tensor("brq", [128, 2], F32, kind="ExternalInput")
    d["bvu"] = nc.dram_tensor("bvu", [1, HPC * D], F32, kind="ExternalInput")
    d["onesd"] = nc.dram_tensor("onesd", [128, 1], F32R,
                                kind="ExternalInput")
    d["cosT"] = nc.dram_tensor("cosT", [128, S], F32, kind="ExternalInput")
    d["sinT"] = nc.dram_tensor("sinT", [128, S], F32, kind="ExternalInput")
    d["out"] = nc.dram_tensor("out", [S, E], F32, kind="ExternalOutput")
    return d


def _consts(nc, tc, top, d):
    """Persistent tiles: K/Q/V storage, biases, ones, up-weights."""
    import concourse.mybir as mybir
    F32 = mybir.dt.float32
    F32R = mybir.dt.float32r

    kq_pool = top.enter_context(tc.tile_pool(name="kq", bufs=1))
    v_pool = top.enter_context(tc.tile_pool(name="vp", bufs=1))
    cpool = top.enter_context(tc.tile_pool(name="cp", bufs=1))

    t = {}
    t["K"] = [kq_pool.tile([128, S], F32R, name=f"Kt{h}") for h in range(HPC)]
    t["Q"] = [kq_pool.tile([128, S], F32R, name=f"Qt{h}") for h in range(HPC)]
    t["V"] = [v_pool.tile([128, HPC * D], F32R, name=f"Vt{i}")
              for i in range(NKC)]

    def ld(name, dram, shape, dt=F32):
        tl = cpool.tile(shape, dt, name=name)
        nc.sync.dma_start(tl[:], dram[:])
        return tl

    t["ones"] = ld("ones_t", d["onesd"], [128, 1], F32R)
    t["bkvd"] = ld("bkvd_t", d["bkvd"], [128, NL])
    t["bqd"] = ld("bqd_t", d["bqd"], [128, NL])
    t["bku"] = ld("bku_t", d["bku"], [128, 2])
    t["bqu"] = ld("bqu_t", d["bqu"], [128, 2])
    t["brk"] = ld("brk_t", d["brk"], [128, 2])
    t["brq"] = ld("brq_t", d["brq"], [128, 2])
    bvu_row = ld("bvu_row", d["bvu"], [1, HPC * D])
    bvu_bc = cpool.tile([128, HPC * D], F32, name="bvu_bc")
    nc.gpsimd.partition_broadcast(bvu_bc[:], bvu_row[:])
    t["bvu_bc"] = bvu_bc

    upw = {}
    for nm, key, w in (("ku", "wkuT", HPC * R), ("qu", "wquT", HPC * R),
                       ("rq", "wrqT", HPC * R), ("vu", "wvuT", HPC * D)):
        tl = [cpool.tile([128, w], F32R, name=f"w{nm}{l}") for l in range(NL)]
        for l in range(NL):
            nc.sync.dma_start(tl[l][:], d[key][l * 128:(l + 1) * 128, :])
        upw[nm] = tl
    t["upw"] = upw
    return t


def _phaseA_pools(tc, pa):
    p = {}
    p["xa"] = pa.enter_context(tc.tile_pool(name="xa", bufs=1))
    p["wa"] = pa.enter_context(tc.tile_pool(name="wa", bufs=2))
    p["kvq"] = pa.enter_context(tc.tile_pool(name="kvq", bufs=1))
    p["cs"] = pa.enter_context(tc.tile_pool(name="cs", bufs=1))
    p["rp"] = pa.enter_context(tc.tile_pool(name="rp", bufs=1))
    p["psA"] = pa.enter_context(tc.tile_pool(name="psA", bufs=3,
                                             space="PSUM"))
    return p


def _emit_A(nc, tc, d, t, p):
    import concourse.mybir as mybir
    from concourse.alu_op_type import AluOpType
    F32 = mybir.dt.float32
    F32R = mybir.dt.float32r
    K_t, Q_t, V_t, upw = t["K"], t["Q"], t["V"], t["upw"]
    swap_mask = [i ^ 1 for i in range(32)]

    for sc in range(NSC):
        ssl = slice(sc * SW, (sc + 1) * SW)
        xt = p["xa"].tile([128, NE * SW], F32R, name="xt")
        nc.sync.dma_start(
            xt[:].rearrange("p (e s) -> p e s", e=NE),
            d["xT"][:, ssl].rearrange("(e p) s -> p e s", p=128))
        cos_s = p["cs"].tile([128, SW], F32, name="cos_s")
        nc.sync.dma_start(cos_s[:], d["cosT"][:, ssl])
        sin_s = p["cs"].tile([128, SW], F32, name="sin_s")
        nc.sync.dma_start(sin_s[:], d["sinT"][:, ssl])

        def down_mm(wdram, m):
            wt = p["wa"].tile([128, NE * 128], F32R, name="wt")
            nc.sync.dma_start(
                wt[:].rearrange("p (e c) -> p e c", e=NE),
                wdram[:, m * 128:(m + 1) * 128].rearrange(
                    "(e p) c -> p e c", p=128))
            ps = p["psA"].tile([128, SW], F32, name="psA_t")
            for e in range(NE):
                nc.tensor.matmul(ps[:], wt[:, e * 128:(e + 1) * 128],
                                 xt[:, e * SW:(e + 1) * SW],
                                 start=(e == 0), stop=(e == NE - 1))
            return ps

        def rope(ps, bias_t, m, dst):
            # ps: [128 rows = 2 heads x 64 rope rows, SW]
            xb = p["rp"].tile([128, SW], F32, name="xb")
            nc.vector.tensor_scalar_add(xb[:], ps[:], bias_t[:, m:m + 1])
            sh = p["rp"].tile([128, SW], F32, name="sh")
            nc.vector.stream_shuffle(sh[:], xb[:], swap_mask)
            t1 = p["rp"].tile([128, SW], F32, name="t1")
            nc.vector.tensor_tensor(t1[:], xb[:], cos_s[:],
                                    op=AluOpType.mult)
            t2 = p["rp"].tile([128, SW], F32, name="t2")
            nc.vector.tensor_tensor(t2[:], sh[:], sin_s[:],
                                    op=AluOpType.mult)
            nc.vector.tensor_tensor(dst[2 * m][R:D, ssl], t1[0:R, :],
                                    t2[0:R, :], op=AluOpType.add)
            nc.vector.tensor_tensor(dst[2 * m + 1][R:D, ssl], t1[R:D, :],
                                    t2[R:D, :], op=AluOpType.add)

        def up_mm(src, w, m):
            ps = p["psA"].tile([128, SW], F32, name="psA_t")
            for l in range(NL):
                nc.tensor.matmul(ps[:], w[l][:, m * 128:(m + 1) * 128],
                                 src[l][:], start=(l == 0),
                                 stop=(l == NL - 1))
            return ps

        # latent kv_d down-projection (replicated in batch group)
        kv_s = []
        for m in range(NL):
            ps = down_mm(d["wkvdT"], m)
            tl = p["kvq"].tile([128, SW], F32R, name=f"lat{m}")
            nc.scalar.add(tl[:], ps[:], t["bkvd"][:, m:m + 1])
            kv_s.append(tl)
        for m in range(2):  # k1 -> K rows 0..63
            ps = up_mm(kv_s, upw["ku"], m)
            nc.vector.tensor_scalar_add(K_t[2 * m][0:R, ssl], ps[0:R, :],
                                        t["bku"][0:R, m:m + 1])
            nc.vector.tensor_scalar_add(K_t[2 * m + 1][0:R, ssl], ps[R:D, :],
                                        t["bku"][R:D, m:m + 1])
        for j in range(SW // 128):  # V, (s, feat) layout
            ps = p["psA"].tile([128, HPC * D], F32, name="psV_t")
            for l in range(NL):
                nc.tensor.matmul(ps[:], kv_s[l][:, j * 128:(j + 1) * 128],
                                 upw["vu"][l][:], start=(l == 0),
                                 stop=(l == NL - 1))
            nc.vector.tensor_tensor(V_t[sc * (SW // 128) + j][:], ps[:],
                                    t["bvu_bc"][:], op=AluOpType.add)

        # latent q_d down-projection (slots shared with kv_s)
        q_s = []
        for m in range(NL):
            ps = down_mm(d["wqdT"], m)
            tl = p["kvq"].tile([128, SW], F32R, name=f"lat{m}")
            nc.scalar.add(tl[:], ps[:], t["bqd"][:, m:m + 1])
            q_s.append(tl)
        for m in range(2):  # q1 -> Q rows 0..63
            ps = up_mm(q_s, upw["qu"], m)
            nc.vector.tensor_scalar_add(Q_t[2 * m][0:R, ssl], ps[0:R, :],
                                        t["bqu"][0:R, m:m + 1])
            nc.vector.tensor_scalar_add(Q_t[2 * m + 1][0:R, ssl], ps[R:D, :],
                                        t["bqu"][R:D, m:m + 1])
        for m in range(2):  # rope-q from q_d
            ps = up_mm(q_s, upw["rq"], m)
            rope(ps, t["brq"], m, Q_t)
        # rope-k from x
        for m in range(2):
            ps = down_mm(d["wrkT"], m)
            rope(ps, t["brk"], m, K_t)


def _phaseB_pools(tc, pb):
    p = {}
    p["pe"] = pb.enter_context(tc.tile_pool(name="pe", bufs=3))
    p["sm"] = pb.enter_context(tc.tile_pool(name="sm", bufs=2))
    p["cb"] = pb.enter_context(tc.tile_pool(name="cb", bufs=2))
    p["psS"] = pb.enter_context(tc.tile_pool(name="psS", bufs=3,
                                             space="PSUM"))
    p["psO"] = pb.enter_context(tc.tile_pool(name="psO", bufs=1,
                                             space="PSUM"))
    return p


def _emit_B(nc, tc, d, t, p, att_t, mode="full"):
    import concourse.mybir as mybir
    from concourse.alu_op_type import AluOpType
    F32 = mybir.dt.float32
    F32R = mybir.dt.float32r
    AF = mybir.ActivationFunctionType
    K_t, Q_t, V_t = t["K"], t["Q"], t["V"]

    LAG = 3  # PV trails QK/exp by LAG k-chunks so PE never waits on ACT

    for h in range(HPC):
        for qp in range(2):
            qa = slice((2 * qp) * 512, (2 * qp + 1) * 512)
            qb = slice((2 * qp + 1) * 512, (2 * qp + 2) * 512)
            oA = p["psO"].tile([128, 512], F32, name="oA")
            oB = p["psO"].tile([128, 512], F32, name="oB")
            # two interleaved row-sum chains so DVE drains overlap
            acc0 = p["pe"].tile([128, 1024], F32R, name="acc0", bufs=1)
            acc1 = p["pe"].tile([128, 1024], F32R, name="acc1", bufs=1)
            accs = (acc0, acc1)
            pes = {}

            def pv(kk):
                pe = pes.pop(kk)
                nc.tensor.matmul(oA[:], V_t[kk][:, h * D:(h + 1) * D],
                                 pe[:, 0:512], start=(kk == 0),
                                 stop=(kk == NKC - 1))
                nc.tensor.matmul(oB[:], V_t[kk][:, h * D:(h + 1) * D],
                                 pe[:, 512:1024], start=(kk == 0),
                                 stop=(kk == NKC - 1))

            for kk in range(NKC):
                ksl = slice(kk * 128, (kk + 1) * 128)
                pp = p["psS"].tile([128, 1024], F32, name="pp")
                nc.tensor.matmul(pp[:, 0:512], K_t[h][:, ksl], Q_t[h][:, qa],
                                 start=True, stop=True)
                nc.tensor.matmul(pp[:, 512:1024], K_t[h][:, ksl],
                                 Q_t[h][:, qb], start=True, stop=True)
                if mode == "qk":
                    continue
                pe = p["pe"].tile([128, 1024], F32R, name="pet", bufs=5)
                nc.scalar.activation(pe[:], pp[:], AF.Exp, scale=SCALE)
                if mode == "qke":
                    continue
                # row-sum accumulation on DVE (keys land on partitions later)
                acc = accs[kk % 2]
                if kk < 2:
                    nc.vector.tensor_copy(acc[:], pe[:])
                else:
                    nc.vector.tensor_tensor(acc[:], pe[:], acc[:],
                                            op=AluOpType.add)
                pes[kk] = pe
                if kk >= LAG:
                    pv(kk - LAG)
            if mode != "full":
                continue
            for kk in range(NKC - LAG, NKC):
                pv(kk)
            # r[q] = sum_p acc[p, q] via ones-matmul; then 1/r broadcast
            nc.vector.tensor_tensor(acc0[:], acc1[:], acc0[:],
                                    op=AluOpType.add)
            sums = p["psS"].tile([1, 1024], F32, name="pp")
            nc.tensor.matmul(sums[:, 0:512], t["ones"][:], acc0[:, 0:512],
                             start=True, stop=True)
            nc.tensor.matmul(sums[:, 512:1024], t["ones"][:],
                             acc0[:, 512:1024], start=True, stop=True)
            rr = p["sm"].tile([1, 1024], F32, name="rr")
            nc.vector.tensor_copy(rr[:], sums[:])
            ci = p["sm"].tile([1, 1024], F32, name="ci")
            nc.vector.reciprocal(ci[:], rr[:])
            cb = p["cb"].tile([128, 1024], F32, name="cbt")
            nc.gpsimd.partition_broadcast(cb[:], ci[:])
            nc.vector.tensor_tensor(att_t[h][:, qa], oA[:], cb[:, 0:512],
                                    op=AluOpType.mult)
            nc.vector.tensor_tensor(att_t[h][:, qb], oB[:], cb[:, 512:1024],
                                    op=AluOpType.mult)


def _phaseC_pools(tc, pc):
    p = {}
    p["wo"] = pc.enter_context(tc.tile_pool(name="wo", bufs=1))
    p["oc"] = pc.enter_context(tc.tile_pool(name="oc", bufs=3))
    p["psC"] = pc.enter_context(tc.tile_pool(name="psC", bufs=3,
                                             space="PSUM"))
    return p


def _emit_C(nc, tc, d, t, p, att_t, wo_t):
    import concourse.mybir as mybir
    F32 = mybir.dt.float32

    for sj in range(S // 128):
        for ocn in range(E // 512):
            ps = p["psC"].tile([128, 512], F32, name="psC_t")
            for hc in range(HPC):
                nc.tensor.matmul(ps[:], att_t[hc][:, sj * 128:(sj + 1) * 128],
                                 wo_t[hc][:, ocn * 512:(ocn + 1) * 512],
                                 start=(hc == 0), stop=(hc == HPC - 1))
            ob = p["oc"].tile([128, 512], F32, name="ob")
            nc.vector.tensor_copy(ob[:], ps[:])
            nc.sync.dma_start(
                d["out"][sj * 128:(sj + 1) * 128,
                         ocn * 512:(ocn + 1) * 512], ob[:])


def _load_wo(nc, pool, d):
    import concourse.mybir as mybir
    F32R = mybir.dt.float32r
    wo_t = [pool.tile([128, E], F32R, name=f"wo{hc}") for hc in range(HPC)]
    for hc in range(HPC):
        nc.sync.dma_start(wo_t[hc][:], d["woT"][hc * 128:(hc + 1) * 128, :])
    return wo_t


def _build_program(loop=None):
    """loop=None: normal kernel. loop=(phase, n): benchmark variant with a
    hardware For_i loop repeating one phase n times."""
    import concourse.bacc as bacc
    import concourse.mybir as mybir
    import concourse.tile as tile

    F32R = mybir.dt.float32r

    nc = bacc.Bacc("TRN2", target_bir_lowering=False, debug=False,
                   num_devices=NCORES)
    d = _mk(nc)

    with tile.TileContext(nc) as tc, ExitStack() as top:
        t = _consts(nc, tc, top, d)
        if loop is None:
            with ExitStack() as pa:
                pA = _phaseA_pools(tc, pa)
                _emit_A(nc, tc, d, t, pA)
            with ExitStack() as pb:
                att_pool = pb.enter_context(tc.tile_pool(name="att", bufs=1))
                att_t = [att_pool.tile([128, S], F32R, name=f"att{h}")
                         for h in range(HPC)]
                with ExitStack() as pbi:
                    pB = _phaseB_pools(tc, pbi)
                    _emit_B(nc, tc, d, t, pB, att_t)
                with ExitStack() as pc:
                    pC = _phaseC_pools(tc, pc)
                    wo_t = _load_wo(nc, pC["wo"], d)
                    _emit_C(nc, tc, d, t, pC, att_t, wo_t)
        else:
            phase, n = loop

            def _fill(tile_, w):
                nc.sync.dma_start(tile_[:], d["xT"][0:128, 0:w])

            with ExitStack() as ps_:
                if phase == "A":
                    pA = _phaseA_pools(tc, ps_)
                    with tc.For_i(0, n, 1):
                        _emit_A(nc, tc, d, t, pA)
                elif phase.startswith("B"):
                    mode = {"B": "full", "B0": "qk", "B1": "qke"}[phase]
                    for h in range(HPC):
                        _fill(t["K"][h], S)
                        _fill(t["Q"][h], S)
                    for i in range(NKC):
                        _fill(t["V"][i], HPC * D)
                    att_pool = ps_.enter_context(
                        tc.tile_pool(name="att", bufs=1))
                    att_t = [att_pool.tile([128, S], F32R, name=f"att{h}")
                             for h in range(HPC)]
                    pB = _phaseB_pools(tc, ps_)
                    with tc.For_i(0, n, 1):
                        _emit_B(nc, tc, d, t, pB, att_t, mode)
                elif phase == "C":
                    att_pool = ps_.enter_context(
                        tc.tile_pool(name="att", bufs=1))
                    att_t = [att_pool.tile([128, S], F32R, name=f"att{h}")
                             for h in range(HPC)]
                    for h in range(HPC):
                        _fill(att_t[h], S)
                    pC = _phaseC_pools(tc, ps_)
                    wo_t = _load_wo(nc, pC["wo"], d)
                    with tc.For_i(0, n, 1):
                        _emit_C(nc, tc, d, t, pC, att_t, wo_t)
                else:
                    raise ValueError(phase)

    nc.compile()
    return nc


def _rope_tables():
    inv_freq = 1.0 / (10000.0 ** (np.arange(0, R, 2, dtype=np.float64) / R))
    t = np.arange(S, dtype=np.float64)
    freqs = np.outer(t, inv_freq)                       # (S, R/2)
    emb = np.concatenate([freqs, freqs], axis=-1)       # (S, R)
    cos = np.cos(emb).astype(np.float32)                # (S, R)
    sin = np.sin(emb).astype(np.float32)
    perm = np.array([(j // 2) if j % 2 == 0 else (j // 2) + R // 2
                     for j in range(R)])
    sign = np.array([-1.0 if j % 2 == 0 else 1.0
                     for j in range(R)], dtype=np.float32)
    cos_p = cos[:, perm].T.copy()                       # (R, S)
    sin_p = (sin[:, perm] * sign[None, :]).T.copy()     # (R, S)
    cosT = np.concatenate([cos_p, cos_p], axis=0)       # (128, S)
    sinT = np.concatenate([sin_p, sin_p], axis=0)
    return cosT, sinT, perm


def _per_core_inputs(inputs, core):
    b, hg = divmod(core, HPC)
    cosT, sinT, perm = _rope_tables()
    hsl64 = np.concatenate([hg * HPC * R + h * R + perm
                            for h in range(HPC)])       # permuted rope rows
    hs64 = slice(hg * HPC * R, (hg + 1) * HPC * R)      # natural 64-rows
    hs128 = slice(hg * HPC * D, (hg + 1) * HPC * D)     # natural 128-rows

    x = np.asarray(inputs["x"], dtype=np.float32)
    f = np.float32
    im = {
        "xT": np.ascontiguousarray(x[b].T),
        "wkvdT": np.ascontiguousarray(np.asarray(inputs["Wkv_d"], f).T),
        "wqdT": np.ascontiguousarray(np.asarray(inputs["Wq_d"], f).T),
        "wrkT": np.ascontiguousarray(np.asarray(inputs["Wrk"], f)[hsl64].T),
        "wkuT": np.ascontiguousarray(np.asarray(inputs["Wk_u"], f)[hs64].T),
        "wquT": np.ascontiguousarray(np.asarray(inputs["Wq_u"], f)[hs64].T),
        "wrqT": np.ascontiguousarray(np.asarray(inputs["Wrq"], f)[hsl64].T),
        "wvuT": np.ascontiguousarray(np.asarray(inputs["Wv_u"], f)[hs128].T),
        "woT": np.ascontiguousarray(np.asarray(inputs["Wo"], f).T[hs128]),
        "bkvd": np.ascontiguousarray(
            np.asarray(inputs["bkv_d"], f).reshape(NL, 128).T),
        "bqd": np.ascontiguousarray(
            np.asarray(inputs["bq_d"], f).reshape(NL, 128).T),
        "bku": np.ascontiguousarray(
            np.asarray(inputs["bk_u"], f)[hs64].reshape(2, 128).T),
        "bqu": np.ascontiguousarray(
            np.asarray(inputs["bq_u"], f)[hs64].reshape(2, 128).T),
        "brk": np.ascontiguousarray(
            np.asarray(inputs["brk"], f)[hsl64].reshape(2, 128).T),
        "brq": np.ascontiguousarray(
            np.asarray(inputs["brq"], f)[hsl64].reshape(2, 128).T),
        "bvu": np.ascontiguousarray(
            np.asarray(inputs["bv_u"], f)[hs128].reshape(1, HPC * D)),
        "onesd": np.ones((128, 1), dtype=np.float32),
        "cosT": cosT,
        "sinT": sinT,
    }
    return im


def _get_runtime(loop=None):
    key = loop
    if key in _RT:
        return _RT[key]
    import jax
    import numpy as _np
    from jax.sharding import Mesh, PartitionSpec
    from jax.experimental.shard_map import shard_map

    import concourse.mybir as mybir
    from concourse import bass2jax

    nc = _build_program(loop)
    bass2jax.install_neuronx_cc_hook()

    partition_name = (nc.partition_id_tensor.name
                      if nc.partition_id_tensor else None)
    in_names, out_names, out_avals, zero_shapes = [], [], [], []
    for alloc in nc.m.functions[0].allocations:
        if not isinstance(alloc, mybir.MemoryLocationSet):
            continue
        name = alloc.memorylocations[0].name
        if alloc.kind == "ExternalInput":
            if name != partition_name:
                in_names.append(name)
        elif alloc.kind == "ExternalOutput":
            out_names.append(name)
            np_dt = mybir.dt.np(alloc.dtype)
            out_avals.append(jax.core.ShapedArray(
                tuple(alloc.tensor_shape), np_dt))
            zero_shapes.append((tuple(alloc.tensor_shape), np_dt))

    n_params = len(in_names)
    n_outs = len(out_names)
    all_in_names = list(in_names) + list(out_names)
    if partition_name is not None:
        all_in_names.append(partition_name)

    def _body(*args):
        operands = list(args)
        if partition_name is not None:
            operands.append(bass2jax.partition_id_tensor())
        outs = bass2jax._bass_exec_p.bind(
            *operands,
            out_avals=tuple(out_avals),
            in_names=tuple(all_in_names),
            out_names=tuple(out_names),
            lowering_input_output_aliases=(),
            sim_require_finite=True,
            sim_require_nnan=True,
            nc=nc,
        )
        return tuple(outs)

    devices = jax.devices()[:NCORES]
    mesh = Mesh(_np.asarray(devices), ("core",))
    in_specs = (PartitionSpec("core"),) * (n_params + n_outs)
    out_specs = (PartitionSpec("core"),) * n_outs
    donate = tuple(range(n_params, n_params + n_outs))
    sharded = jax.jit(
        shard_map(_body, mesh=mesh, in_specs=in_specs, out_specs=out_specs,
                  check_rep=False),
        donate_argnums=donate, keep_unused=True)

    _RT[key] = dict(sharded=sharded, in_names=in_names, out_names=out_names,
                    zero_shapes=zero_shapes, n_outs=n_outs)
    return _RT[key]


def _run_cores(in_maps):
    rt = _get_runtime()
    import numpy as _np
    concat_in = [
        _np.concatenate([in_maps[c][name] for c in range(NCORES)], axis=0)
        for name in rt["in_names"]
    ]
    concat_zeros = [
        _np.zeros((NCORES * shp[0],) + shp[1:], dt)
        for (shp, dt) in rt["zero_shapes"]
    ]
    out_arrs = rt["sharded"](*concat_in, *concat_zeros)
    res = []
    for c in range(NCORES):
        m = {}
        for i, name in enumerate(rt["out_names"]):
            shp, dt = rt["zero_shapes"][i]
            m[name] = _np.asarray(out_arrs[i]).reshape((NCORES,) + shp)[c]
        res.append(m)
    return res


def kernel(**inputs):
    in_maps = [_per_core_inputs(inputs, c) for c in range(NCORES)]
    res = _run_cores(in_maps)
    bo = np.asarray(inputs["bo"], dtype=np.float32)
    final = np.empty((B, S, E), dtype=np.float32)
    for b in range(B):
        acc = res[HPC * b]["out"].astype(np.float32).copy()
        for g in range(1, HPC):
            acc += res[HPC * b + g]["out"]
        final[b] = acc + bo[None, :]
    return final

